# revision 1
# baseline (speedup 1.0000x reference)
import os
import numpy as np

import concourse.bacc as bacc
import concourse.mybir as mybir
import concourse.tile as tile
from concourse.bass_utils import run_bass_kernel_spmd

# Problem constants (hardcoded per harness contract)
B, H, W, C = 32, 32, 32, 128
NUM, D0, D1 = 10, 60, 16
JK = D0 * D1            # 960
OO = NUM * JK           # 9600
P = H * W               # 1024 contraction dim of the dense kernel
N_CORES = 8
B_LOC = B // N_CORES    # 4 batches per core
BLK = 384               # dense-kernel column block (>=256 keeps fp32r at 1 cyc/row)
NBLK = OO // BLK        # 25
EPS = 1e-12

f32 = mybir.dt.float32
f32r = mybir.dt.float32r
AF = mybir.ActivationFunctionType
ALU = mybir.AluOpType


def build_nc():
    nc = bacc.Bacc("TRN2", debug=False)
    u_d = nc.dram_tensor("u", (B_LOC, H, W, C), f32, kind="ExternalInput").ap()
    wc_d = nc.dram_tensor("wc", (2, 2, C, C), f32, kind="ExternalInput").ap()
    km_d = nc.dram_tensor("km", (P, OO), f32, kind="ExternalInput").ap()
    eye_d = nc.dram_tensor("eye", (C, C), f32, kind="ExternalInput").ap()
    out_d = nc.dram_tensor("out", (B_LOC, NUM, JK), f32, kind="ExternalOutput").ap()

    with tile.TileContext(nc) as tc:
        with tc.tile_pool(name="persist", bufs=1) as pers:
            u_hat = pers.tile([128, B_LOC, OO], f32r)       # [n, b, o]
            uT = pers.tile([128, B_LOC, 8, 128], f32r)      # lhsT chunks [p, b, chunk, c]
            wct = pers.tile([128, 4, C], f32r)              # conv taps [ci, tap, co]
            eye = pers.tile([128, C], f32r)
            ones = pers.tile([128, 128], f32)
            crep0 = pers.tile([128, 128], f32r)             # uniform c = 0.1 (softmax of zeros)
            c_all = pers.tile([128, B_LOC, NUM], f32)
            z_all = pers.tile([128, B_LOC * NUM], f32)
            ss_all = pers.tile([128, B_LOC * NUM], f32)
            alpha = pers.tile([128, B_LOC * NUM], f32)
            blog = pers.tile([128, B_LOC * NUM], f32)
            eexp = pers.tile([128, B_LOC, NUM], f32)
            nmax = pers.tile([128, B_LOC], f32)
            sume = pers.tile([128, B_LOC], f32)
            rsum = pers.tile([128, B_LOC], f32)

            xpad = pers.tile([128, 33 * 33], f32r)
            zcol = pers.tile([128, 33], f32)

            nc.gpsimd.dma_start(wct[:], wc_d.rearrange("dh dw ci co -> ci (dh dw) co"))
            nc.gpsimd.dma_start(eye[:], eye_d)
            nc.vector.memset(ones[:], 1.0)
            nc.vector.memset(zcol[:], 0.0)
            nc.vector.tensor_scalar_mul(crep0[:], ones[:], 0.1)
            xpad_v = xpad[:].rearrange("p (h w) -> p h w", w=33)
            nc.vector.tensor_copy(xpad_v[:, :, 32], zcol[:])   # right pad col
            nc.vector.tensor_copy(xpad_v[:, 32, :], zcol[:])   # bottom pad row

            # ---------- Phase 1: 2x2 SAME conv, per batch ----------
            # out[co, s=h*32+w] = sum_taps Wtap.T @ xpad[:, (h+dh)*33 + (w+dw)]
            with tc.tile_pool(name="convp", bufs=2) as cvp, \
                 tc.tile_pool(name="psc", bufs=2, space="PSUM") as psc, \
                 tc.tile_pool(name="pst", bufs=2, space="PSUM") as pst:
                for b in range(B_LOC):
                    xin = cvp.tile([128, 8, 128], f32r, tag="xin")
                    nc.gpsimd.dma_start(
                        xin[:],
                        u_d[b].rearrange("h w c -> (h w) c").rearrange(
                            "(t sp) c -> sp t c", sp=128))
                    for t in range(8):
                        pt = pst.tile([128, 128], f32r, tag="pt")
                        nc.tensor.transpose(pt[:], xin[:, t, :], eye[:])
                        # pt[ch, sp] covers s = t*128 + sp -> rows h = t*4..t*4+4
                        src = pt[:].rearrange("p (a w) -> p a w", w=32)
                        dst = xpad_v[:, t * 4:(t + 1) * 4, 0:32]
                        if t % 2 == 0:
                            nc.vector.tensor_copy(dst, src)
                        else:
                            nc.scalar.copy(dst, src)
                    for hh in range(2):
                        pc = psc.tile([128, 512], f32, tag="pc")
                        for ti, (dh, dw) in enumerate(((0, 0), (0, 1), (1, 0), (1, 1))):
                            rhs = xpad_v[:, hh * 16 + dh: hh * 16 + dh + 16, dw:dw + 32]
                            nc.tensor.matmul(pc[:], wct[:, ti, :], rhs,
                                             start=(ti == 0), stop=(ti == 3))
                        # raw-reshape gather: uT[t][pp, c] = conv[a, 8q+t, pp], c = 4a+q
                        pcv = pc[:].rearrange("p (a q t) -> p a q t", q=4, t=8)
                        for t in range(8):
                            src = pcv[:, :, :, t]
                            dst = uT[:, b, t, hh * 64:(hh + 1) * 64].rearrange(
                                "p (a q) -> p a q", q=4)
                            if t % 2 == 0:
                                nc.vector.tensor_copy(dst, src)
                            else:
                                nc.scalar.copy(dst, src)

            # ---------- Phase 2: dense matmul u_hat = uT.T @ km ----------
            NPH = int(os.environ.get("KPHASES", "3"))
            with tc.tile_pool(name="kp", bufs=2) as kp, \
                 tc.tile_pool(name="psm", bufs=3, space="PSUM") as psm:
                kv = km_d.rearrange("(c p) o -> p c o", p=128)
                for blk in range(NBLK if NPH >= 2 else 0):
                    kt = kp.tile([128, 8, BLK], f32r, tag="kt")
                    nc.gpsimd.dma_start(kt[:], kv[:, :, blk * BLK:(blk + 1) * BLK])
                    for b in range(B_LOC):
                        pm = psm.tile([128, BLK], f32, tag="pm")
                        for ch in range(8):
                            nc.tensor.matmul(pm[:], uT[:, b, ch, :], kt[:, ch, :],
                                             start=(ch == 0), stop=(ch == 7))
                        dst = u_hat[:, b, blk * BLK:(blk + 1) * BLK]
                        if (blk * B_LOC + b) % 2 == 0:
                            nc.vector.tensor_copy(dst, pm[:])
                        else:
                            nc.scalar.copy(dst, pm[:])

            # ---------- Phase 3: dynamic routing (3 iterations) ----------
            with tc.tile_pool(name="rt", bufs=2) as rt, \
                 tc.tile_pool(name="psb", bufs=3, space="PSUM") as psb:
                KR = int(os.environ.get("KROUT", "5"))
                for it in range((3 if KR >= 5 else 1) if NPH >= 3 else 0):
                    for b in range(B_LOC):
                        for i in range(NUM):
                            if it == 0:
                                crep = crep0
                            else:
                                crep = rt.tile([128, 128], f32r, tag="crep")
                                nc.vector.tensor_scalar_mul(
                                    crep[:], ones[:], c_all[:, b, i:i + 1])
                            pbc = psb.tile([128, JK], f32, tag="pbc")
                            o0 = i * JK
                            nc.tensor.matmul(pbc[:, 0:512], crep[:],
                                             u_hat[:, b, o0:o0 + 512],
                                             start=True, stop=True)
                            nc.tensor.matmul(pbc[:, 512:JK], crep[:],
                                             u_hat[:, b, o0 + 512:o0 + JK],
                                             start=True, stop=True)
                            if it < 2 and KR >= 2:
                                un = b * NUM + i
                                scr = rt.tile([128, JK], f32, tag="scr")
                                scr2 = rt.tile([128, JK], f32, tag="scr2")
                                nc.vector.scalar_tensor_tensor(
                                    out=scr[:],
                                    in0=u_hat[:, b, o0:o0 + JK].bitcast(f32),
                                    scalar=1.0, in1=pbc[:],
                                    op0=ALU.mult, op1=ALU.mult,
                                    accum_out=z_all[:, un:un + 1])
                                if KR >= 3:
                                    nc.scalar.activation(
                                        scr2[:], pbc[:], AF.Square,
                                        accum_out=ss_all[:, un:un + 1])
                                else:
                                    nc.vector.memset(ss_all[:, un:un + 1], 1.0)
                            else:
                                ofin = rt.tile([1, JK], f32, tag="ofin")
                                if (b * NUM + i) % 2 == 0:
                                    nc.vector.tensor_copy(ofin[:], pbc[0:1, :])
                                else:
                                    nc.scalar.copy(ofin[:], pbc[0:1, :])
                                nc.sync.dma_start(out_d[b, i], ofin[:])
                    if it < 2 and KR >= 4:
                        # b_logits = z * rsqrt(max(ss, eps)); then softmax over capsules
                        nc.vector.tensor_scalar_max(ss_all[:], ss_all[:], EPS)
                        nc.scalar.activation(ss_all[:], ss_all[:], AF.Sqrt)
                        nc.vector.reciprocal(alpha[:], ss_all[:])
                        nc.vector.tensor_mul(blog[:], z_all[:], alpha[:])
                        blv = blog[:].rearrange("p (b i) -> p b i", i=NUM)
                        nc.vector.tensor_reduce(nmax[:], blv, axis=mybir.AxisListType.X,
                                                op=ALU.max, negate=True)
                        for b in range(B_LOC):
                            nc.scalar.activation(eexp[:, b, :], blv[:, b, :], AF.Exp,
                                                 bias=nmax[:, b:b + 1],
                                                 accum_out=sume[:, b:b + 1])
                        nc.vector.reciprocal(rsum[:], sume[:])
                        for b in range(B_LOC):
                            nc.vector.tensor_scalar_mul(
                                c_all[:, b, :], eexp[:, b, :], rsum[:, b:b + 1])
    nc.compile()
    return nc


_NC_CACHE = None


def _get_nc():
    global _NC_CACHE
    if _NC_CACHE is None:
        _NC_CACHE = build_nc()
    return _NC_CACHE


def kernel(u_vecs, W_conv, kernel):
    u_vecs = np.ascontiguousarray(np.asarray(u_vecs, dtype=np.float32))
    W_conv = np.ascontiguousarray(np.asarray(W_conv, dtype=np.float32))
    km = np.ascontiguousarray(np.asarray(kernel, dtype=np.float32))
    eye = np.eye(C, dtype=np.float32)
    nc = _get_nc()
    in_maps = [
        {"u": u_vecs[ci * B_LOC:(ci + 1) * B_LOC], "wc": W_conv, "km": km, "eye": eye}
        for ci in range(N_CORES)
    ]
    res = run_bass_kernel_spmd(nc, in_maps, core_ids=list(range(N_CORES)))
    out = np.concatenate([r["out"] for r in res.results], axis=0)
    return out.reshape(B, NUM, D0, D1).astype(np.float32)



# revision 12
# speedup vs baseline: 1.2243x; 1.2243x over previous
import numpy as np
import ml_dtypes

import concourse.bacc as bacc
import concourse.mybir as mybir
import concourse.tile as tile
from concourse.bass_utils import run_bass_kernel_spmd

# Problem constants (hardcoded per harness contract)
B, H, W, C = 32, 32, 32, 128
NUM, D0, D1 = 10, 60, 16
JK = D0 * D1            # 960
OO = NUM * JK           # 9600
P = H * W               # 1024 contraction dim of the dense kernel
N_CORES = 8
B_LOC = B // N_CORES    # 4 batches per core
BLK = 480               # dense-kernel column block (2 blocks per capsule)
NBLK = OO // BLK        # 20
EPS = 1e-12

f32 = mybir.dt.float32
bf16 = mybir.dt.bfloat16
AF = mybir.ActivationFunctionType
ALU = mybir.AluOpType
BF = ml_dtypes.bfloat16
C0_BF = float(np.float32(BF(0.1)))  # the exact bf16 value streamed for it=0


def build_nc():
    nc = bacc.Bacc("TRN2", debug=False)
    u_d = nc.dram_tensor("u", (B_LOC, H, W, C), bf16, kind="ExternalInput").ap()
    wc_d = nc.dram_tensor("wc", (2, 2, C, C), bf16, kind="ExternalInput").ap()
    km_d = nc.dram_tensor("km", (P, OO), bf16, kind="ExternalInput").ap()
    eye_d = nc.dram_tensor("eye", (C, C), bf16, kind="ExternalInput").ap()
    out_d = nc.dram_tensor("out", (B_LOC, NUM, JK), f32, kind="ExternalOutput").ap()

    NBI = B_LOC * NUM  # 40 (b,i) pairs

    with tile.TileContext(nc) as tc:
        with tc.tile_pool(name="persist", bufs=1) as pers:
            u_hat = pers.tile([128, B_LOC, OO], bf16)       # [c, b, o]
            uT = pers.tile([128, B_LOC, 8, 128], bf16)      # lhsT chunks [p, b, chunk, c]
            wct = pers.tile([128, 4, C], bf16)              # conv taps [ci, tap, co]
            eye = pers.tile([128, C], bf16)
            ones_bf = pers.tile([128, 128], bf16)
            onecol = pers.tile([128, 1], f32)               # ss reduction lhsT
            onerow = pers.tile([1, 128], f32)               # partition-broadcast lhsT
            crep0 = pers.tile([128, 128], bf16)             # uniform c (softmax of zeros)
            c_all = pers.tile([128, B_LOC, NUM], f32)
            z_all = pers.tile([128, NBI], f32)
            cz = pers.tile([128, NBI], f32)
            ssm = pers.tile([1, NBI], f32)
            alpha = pers.tile([1, NBI], f32)
            blog = pers.tile([128, NBI], f32)
            eexp = pers.tile([128, B_LOC, NUM], f32)
            nmax = pers.tile([128, B_LOC], f32)
            sume = pers.tile([128, B_LOC], f32)
            rsum = pers.tile([128, B_LOC], f32)
            scr = pers.tile([128, JK], bf16)                # STT dump (value unused)

            xpad = pers.tile([128, 33, 34], bf16)
            zcol = pers.tile([128, 34], bf16)

            nc.gpsimd.dma_start(wct[:], wc_d.rearrange("dh dw ci co -> ci (dh dw) co"))
            nc.gpsimd.dma_start(eye[:], eye_d)
            nc.vector.memset(ones_bf[:], 1.0)
            nc.vector.memset(onecol[:], 1.0)
            nc.vector.memset(onerow[:], 1.0)
            nc.vector.memset(zcol[:], 0.0)
            nc.vector.memset(crep0[:], 0.1)
            nc.vector.tensor_copy(xpad[:, :, 32], zcol[:, 0:33])   # right pad col
            nc.vector.tensor_copy(xpad[:, :, 33], zcol[:, 0:33])
            nc.vector.tensor_copy(xpad[:, 32, :], zcol[:])         # bottom pad row

            # ---------- Phase 1: 2x2 SAME conv, per batch ----------
            with tc.tile_pool(name="convp", bufs=2) as cvp, \
                 tc.tile_pool(name="psc", bufs=2, space="PSUM") as psc, \
                 tc.tile_pool(name="pst", bufs=2, space="PSUM") as pst:
                for b in range(B_LOC):
                    xin = cvp.tile([128, 8, 128], bf16, tag="xin")
                    nc.gpsimd.dma_start(
                        xin[:],
                        u_d[b].rearrange("h w c -> (h w) c").rearrange(
                            "(t sp) c -> sp t c", sp=128))
                    for t in range(8):
                        pt = pst.tile([128, 128], bf16, tag="pt")
                        nc.tensor.transpose(pt[:], xin[:, t, :], eye[:])
                        # pt[ch, sp] covers s = t*128 + sp -> rows h = t*4..t*4+4
                        src = pt[:].rearrange("p (a w) -> p a w", w=32)
                        dst = xpad[:, t * 4:(t + 1) * 4, 0:32]
                        if t % 2 == 0:
                            nc.vector.tensor_copy(dst, src)
                        else:
                            nc.scalar.copy(dst, src)
                    for hh in range(2):
                        pc = psc.tile([128, 512], f32, tag="pc")
                        for ti, (dh, dw) in enumerate(((0, 0), (0, 1), (1, 0), (1, 1))):
                            rhs = xpad[:, hh * 16 + dh: hh * 16 + dh + 16, dw:dw + 32]
                            nc.tensor.matmul(pc[:], wct[:, ti, :], rhs,
                                             start=(ti == 0), stop=(ti == 3))
                        # raw-reshape gather: uT[t][pp, c] = conv[a, 8q+t, pp], c = 4a+q
                        src = pc[:].rearrange("p (a q t) -> p t a q", q=4, t=8)
                        dst = uT[:, b, :, hh * 64:(hh + 1) * 64].rearrange(
                            "p t (a q) -> p t a q", q=4)
                        if hh == 0:
                            nc.vector.tensor_copy(dst, src)
                        else:
                            nc.scalar.copy(dst, src)

            # ---------- Phase 2: dense matmul + routing it=0 interleaved ----------
            kv = km_d.rearrange("(c p) o -> p c o", p=128)
            with tc.tile_pool(name="kp", bufs=2) as kp, \
                 tc.tile_pool(name="rt", bufs=2) as rt, \
                 tc.tile_pool(name="psm", bufs=2, space="PSUM") as psm, \
                 tc.tile_pool(name="psb", bufs=2, space="PSUM") as psb, \
                 tc.tile_pool(name="pss", bufs=1, space="PSUM") as pss:
                for blk in range(NBLK):
                    kt = kp.tile([128, 8, BLK], bf16, tag="kt")
                    nc.gpsimd.dma_start(kt[:], kv[:, :, blk * BLK:(blk + 1) * BLK])
                    for b in range(B_LOC):
                        pm = psm.tile([128, BLK], f32, tag="pm")
                        for ch in range(8):
                            nc.tensor.matmul(pm[:], uT[:, b, ch, :], kt[:, ch, :],
                                             start=(ch == 0), stop=(ch == 7))
                        dst = u_hat[:, b, blk * BLK:(blk + 1) * BLK]
                        if (blk * B_LOC + b) % 2 == 0:
                            nc.vector.tensor_copy(dst, pm[:])
                        else:
                            nc.scalar.copy(dst, pm[:])
                    if blk % 2 == 1:
                        # capsule i fully materialized -> run it=0 stream + z on gpsimd
                        i = blk // 2
                        for b in range(B_LOC):
                            pbc = psb.tile([128, JK], f32, tag="pbc")
                            o0 = i * JK
                            nc.tensor.matmul(pbc[:, 0:512], crep0[:],
                                             u_hat[:, b, o0:o0 + 512],
                                             start=True, stop=True)
                            nc.tensor.matmul(pbc[:, 512:JK], crep0[:],
                                             u_hat[:, b, o0 + 512:o0 + JK],
                                             start=True, stop=True)
                            un = b * NUM + i
                            if un % 4 != 3:
                                obf = rt.tile([128, JK], bf16, tag="obf", bufs=3)
                                nc.scalar.copy(obf[:], pbc[:])
                                nc.vector.scalar_tensor_tensor(
                                    out=scr[:], in0=u_hat[:, b, o0:o0 + JK],
                                    scalar=1.0, in1=obf[:],
                                    op0=ALU.mult, op1=ALU.mult,
                                    accum_out=z_all[:, un:un + 1])
                            else:
                                nc.vector.scalar_tensor_tensor(
                                    out=scr[:], in0=u_hat[:, b, o0:o0 + JK],
                                    scalar=1.0, in1=pbc[:],
                                    op0=ALU.mult, op1=ALU.mult,
                                    accum_out=z_all[:, un:un + 1])

                # ---- it0 softmax: ss = c.z, alpha = rsqrt(ss), c = softmax(z*alpha)
                def softmax_update(it):
                    if it == 0:
                        nc.vector.tensor_scalar_mul(cz[:], z_all[:], C0_BF)
                    else:
                        nc.vector.tensor_tensor(
                            cz[:], c_all[:].rearrange("p b i -> p (b i)"), z_all[:],
                            op=ALU.mult)
                    ps_ss = pss.tile([1, NBI], f32, tag="ss")
                    nc.tensor.matmul(ps_ss[:], onecol[:], cz[:], start=True, stop=True)
                    nc.vector.tensor_scalar_max(ssm[:], ps_ss[:], EPS)
                    nc.scalar.activation(ssm[:], ssm[:], AF.Sqrt)
                    nc.vector.reciprocal(alpha[:], ssm[:])
                    ps_al = pss.tile([128, NBI], f32, tag="al")
                    nc.tensor.matmul(ps_al[:], onerow[:], alpha[:], start=True, stop=True)
                    nc.vector.tensor_tensor(blog[:], z_all[:], ps_al[:], op=ALU.mult)
                    blv = blog[:].rearrange("p (b i) -> p b i", i=NUM)
                    nc.vector.tensor_reduce(nmax[:], blv, axis=mybir.AxisListType.X,
                                            op=ALU.max, negate=True)
                    for b in range(B_LOC):
                        nc.scalar.activation(eexp[:, b, :], blv[:, b, :], AF.Exp,
                                             bias=nmax[:, b:b + 1],
                                             accum_out=sume[:, b:b + 1])
                    nc.vector.reciprocal(rsum[:], sume[:])
                    for b in range(B_LOC):
                        nc.vector.tensor_scalar_mul(
                            c_all[:, b, :], eexp[:, b, :], rsum[:, b:b + 1])

                softmax_update(0)

                # ---- it1: streams + z split across vector/gpsimd ----
                for b in range(B_LOC):
                    for i in range(NUM):
                        crep = rt.tile([128, 128], bf16, tag="crep")
                        nc.gpsimd.tensor_scalar_mul(
                            crep[:], ones_bf[:], c_all[:, b, i:i + 1])
                        pbc = psb.tile([128, JK], f32, tag="pbc")
                        o0 = i * JK
                        nc.tensor.matmul(pbc[:, 0:512], crep[:],
                                         u_hat[:, b, o0:o0 + 512],
                                         start=True, stop=True)
                        nc.tensor.matmul(pbc[:, 512:JK], crep[:],
                                         u_hat[:, b, o0 + 512:o0 + JK],
                                         start=True, stop=True)
                        un = b * NUM + i
                        if un % 4 != 3:
                            obf = rt.tile([128, JK], bf16, tag="obf", bufs=3)
                            nc.scalar.copy(obf[:], pbc[:])
                            nc.vector.scalar_tensor_tensor(
                                out=scr[:], in0=u_hat[:, b, o0:o0 + JK],
                                scalar=1.0, in1=obf[:],
                                op0=ALU.mult, op1=ALU.mult,
                                accum_out=z_all[:, un:un + 1])
                        else:
                            nc.vector.scalar_tensor_tensor(
                                out=scr[:], in0=u_hat[:, b, o0:o0 + JK],
                                scalar=1.0, in1=pbc[:],
                                op0=ALU.mult, op1=ALU.mult,
                                accum_out=z_all[:, un:un + 1])

                softmax_update(1)

                # ---- it2: final streams -> DMA out ----
                for b in range(B_LOC):
                    for i in range(NUM):
                        crep = rt.tile([128, 128], bf16, tag="crep")
                        nc.gpsimd.tensor_scalar_mul(
                            crep[:], ones_bf[:], c_all[:, b, i:i + 1])
                        pbc = psb.tile([128, JK], f32, tag="pbc")
                        o0 = i * JK
                        nc.tensor.matmul(pbc[:, 0:512], crep[:],
                                         u_hat[:, b, o0:o0 + 512],
                                         start=True, stop=True)
                        nc.tensor.matmul(pbc[:, 512:JK], crep[:],
                                         u_hat[:, b, o0 + 512:o0 + JK],
                                         start=True, stop=True)
                        ofin = rt.tile([1, JK], f32, tag="ofin")
                        if (b * NUM + i) % 2 == 0:
                            nc.vector.tensor_copy(ofin[:], pbc[0:1, :])
                        else:
                            nc.scalar.copy(ofin[:], pbc[0:1, :])
                        nc.sync.dma_start(out_d[b, i], ofin[:])
    nc.compile()
    return nc


_NC_CACHE = None


def _get_nc():
    global _NC_CACHE
    if _NC_CACHE is None:
        _NC_CACHE = build_nc()
    return _NC_CACHE


def _prep_inputs(u_vecs, W_conv, kernel):
    u_bf = np.ascontiguousarray(np.asarray(u_vecs, dtype=np.float32)).astype(BF)
    wc_bf = np.ascontiguousarray(np.asarray(W_conv, dtype=np.float32)).astype(BF)
    km_bf = np.ascontiguousarray(np.asarray(kernel, dtype=np.float32)).astype(BF)
    eye = np.eye(C, dtype=np.float32).astype(BF)
    return u_bf, wc_bf, km_bf, eye


def kernel(u_vecs, W_conv, kernel):
    u_bf, wc_bf, km_bf, eye = _prep_inputs(u_vecs, W_conv, kernel)
    nc = _get_nc()
    in_maps = [
        {"u": u_bf[ci * B_LOC:(ci + 1) * B_LOC], "wc": wc_bf, "km": km_bf, "eye": eye}
        for ci in range(N_CORES)
    ]
    res = run_bass_kernel_spmd(nc, in_maps, core_ids=list(range(N_CORES)))
    out = np.concatenate([r["out"] for r in res.results], axis=0)
    return out.reshape(B, NUM, D0, D1).astype(np.float32)


# revision 13
# speedup vs baseline: 1.4978x; 1.2234x over previous
import numpy as np
import ml_dtypes

import concourse.bacc as bacc
import concourse.mybir as mybir
import concourse.tile as tile
from concourse.bass_utils import run_bass_kernel_spmd

# Problem constants (hardcoded per harness contract)
B, H, W, C = 32, 32, 32, 128
NUM, D0, D1 = 10, 60, 16
JK = D0 * D1            # 960
OO = NUM * JK           # 9600
P = H * W               # 1024 contraction dim of the dense kernel
N_CORES = 8
B_LOC = B // N_CORES    # 4 batches per core
BLK = 480               # dense-kernel column block (2 blocks per capsule)
NBLK = OO // BLK        # 20
EPS = 1e-12

f32 = mybir.dt.float32
bf16 = mybir.dt.bfloat16
AF = mybir.ActivationFunctionType
ALU = mybir.AluOpType
BF = ml_dtypes.bfloat16
C0_BF = float(np.float32(BF(0.1)))  # the exact bf16 value streamed for it=0


def build_nc():
    nc = bacc.Bacc("TRN2", debug=False)
    u_d = nc.dram_tensor("u", (B_LOC, H, W, C), bf16, kind="ExternalInput").ap()
    wc_d = nc.dram_tensor("wc", (2, 2, C, C), bf16, kind="ExternalInput").ap()
    km_d = nc.dram_tensor("km", (P, OO), bf16, kind="ExternalInput").ap()
    eye_d = nc.dram_tensor("eye", (C, C), bf16, kind="ExternalInput").ap()
    out_d = nc.dram_tensor("out", (B_LOC, NUM, JK), f32, kind="ExternalOutput").ap()

    NBI = B_LOC * NUM  # 40 (b,i) pairs

    with tile.TileContext(nc) as tc:
        with tc.tile_pool(name="persist", bufs=1) as pers:
            u_hat = pers.tile([128, B_LOC, OO], bf16)       # [c, b, o]
            uT = pers.tile([128, B_LOC, 8, 128], bf16)      # lhsT chunks [p, b, chunk, c]
            wct = pers.tile([128, 4, C], bf16)              # conv taps [ci, tap, co]
            eye = pers.tile([128, C], bf16)
            ones_bf = pers.tile([128, 128], bf16)
            onecol = pers.tile([128, 1], f32)               # ss reduction lhsT
            onerow = pers.tile([1, 128], f32)               # partition-broadcast lhsT
            crep0 = pers.tile([128, 128], bf16)             # uniform c (softmax of zeros)
            c_all = pers.tile([128, B_LOC, NUM], f32)
            z_all = pers.tile([128, NBI], f32)
            cz = pers.tile([128, NBI], f32)
            ssm = pers.tile([1, NBI], f32)
            alpha = pers.tile([1, NBI], f32)
            blog = pers.tile([128, NBI], f32)
            eexp = pers.tile([128, B_LOC, NUM], f32)
            nmax = pers.tile([128, B_LOC], f32)
            sume = pers.tile([128, B_LOC], f32)
            rsum = pers.tile([128, B_LOC], f32)
            scr = pers.tile([128, JK], bf16)                # STT dump (value unused)

            xpad = pers.tile([128, 33, 34], bf16)
            zcol = pers.tile([128, 34], bf16)

            nc.gpsimd.dma_start(wct[:], wc_d.rearrange("dh dw ci co -> ci (dh dw) co"))
            nc.gpsimd.dma_start(eye[:], eye_d)
            nc.vector.memset(ones_bf[:], 1.0)
            nc.vector.memset(onecol[:], 1.0)
            nc.vector.memset(onerow[:], 1.0)
            nc.vector.memset(zcol[:], 0.0)
            nc.vector.memset(crep0[:], 0.1)
            nc.vector.tensor_copy(xpad[:, :, 32], zcol[:, 0:33])   # right pad col
            nc.vector.tensor_copy(xpad[:, :, 33], zcol[:, 0:33])
            nc.vector.tensor_copy(xpad[:, 32, :], zcol[:])         # bottom pad row

            # ---------- Phase 1: 2x2 SAME conv, per batch ----------
            with tc.tile_pool(name="convp", bufs=2) as cvp, \
                 tc.tile_pool(name="psc", bufs=2, space="PSUM") as psc, \
                 tc.tile_pool(name="pst", bufs=2, space="PSUM") as pst:
                for b in range(B_LOC):
                    xin = cvp.tile([128, 8, 128], bf16, tag="xin")
                    nc.gpsimd.dma_start(
                        xin[:],
                        u_d[b].rearrange("h w c -> (h w) c").rearrange(
                            "(t sp) c -> sp t c", sp=128))
                    for t in range(8):
                        pt = pst.tile([128, 128], bf16, tag="pt")
                        nc.tensor.transpose(pt[:], xin[:, t, :], eye[:])
                        # pt[ch, sp] covers s = t*128 + sp -> rows h = t*4..t*4+4
                        src = pt[:].rearrange("p (a w) -> p a w", w=32)
                        dst = xpad[:, t * 4:(t + 1) * 4, 0:32]
                        if t % 2 == 0:
                            nc.vector.tensor_copy(dst, src)
                        else:
                            nc.scalar.copy(dst, src)
                    for hh in range(2):
                        pc = psc.tile([128, 512], f32, tag="pc")
                        for ti, (dh, dw) in enumerate(((0, 0), (0, 1), (1, 0), (1, 1))):
                            rhs = xpad[:, hh * 16 + dh: hh * 16 + dh + 16, dw:dw + 32]
                            nc.tensor.matmul(pc[:], wct[:, ti, :], rhs,
                                             start=(ti == 0), stop=(ti == 3))
                        # raw-reshape gather: uT[t][pp, c] = conv[a, 8q+t, pp], c = 4a+q
                        src = pc[:].rearrange("p (a q t) -> p t a q", q=4, t=8)
                        dst = uT[:, b, :, hh * 64:(hh + 1) * 64].rearrange(
                            "p t (a q) -> p t a q", q=4)
                        if hh == 0:
                            nc.vector.tensor_copy(dst, src)
                        else:
                            nc.scalar.copy(dst, src)

            # ---------- Phase 2: dense matmul + routing it=0 interleaved ----------
            kv = km_d.rearrange("(c p) o -> p c o", p=128)
            with tc.tile_pool(name="kp", bufs=2) as kp, \
                 tc.tile_pool(name="rt", bufs=2) as rt, \
                 tc.tile_pool(name="psm", bufs=2, space="PSUM") as psm, \
                 tc.tile_pool(name="psb", bufs=2, space="PSUM") as psb, \
                 tc.tile_pool(name="pss", bufs=1, space="PSUM") as pss:
                for blk in range(NBLK):
                    kt = kp.tile([128, 8, BLK], bf16, tag="kt")
                    nc.gpsimd.dma_start(kt[:], kv[:, :, blk * BLK:(blk + 1) * BLK])
                    for b in range(B_LOC):
                        pm = psm.tile([128, BLK], f32, tag="pm")
                        for ch in range(8):
                            nc.tensor.matmul(pm[:], uT[:, b, ch, :], kt[:, ch, :],
                                             start=(ch == 0), stop=(ch == 7))
                        dst = u_hat[:, b, blk * BLK:(blk + 1) * BLK]
                        if (blk * B_LOC + b) % 2 == 0:
                            nc.vector.tensor_copy(dst, pm[:])
                        else:
                            nc.scalar.copy(dst, pm[:])
                    if blk % 2 == 1:
                        # capsule i fully materialized -> run it=0 stream + z on gpsimd
                        i = blk // 2
                        for b in range(B_LOC):
                            pbc = psb.tile([128, JK], f32, tag="pbc")
                            o0 = i * JK
                            nc.tensor.matmul(pbc[:, 0:512], crep0[:],
                                             u_hat[:, b, o0:o0 + 512],
                                             start=True, stop=True)
                            nc.tensor.matmul(pbc[:, 512:JK], crep0[:],
                                             u_hat[:, b, o0 + 512:o0 + JK],
                                             start=True, stop=True)
                            un = b * NUM + i
                            obf = rt.tile([128, JK], bf16, tag="obf", bufs=4)
                            nc.scalar.copy(obf[:], pbc[:])
                            nc.vector.scalar_tensor_tensor(
                                out=scr[:], in0=u_hat[:, b, o0:o0 + JK],
                                scalar=1.0, in1=obf[:],
                                op0=ALU.mult, op1=ALU.mult,
                                accum_out=z_all[:, un:un + 1])

                # ---- softmax: ss = c.z, alpha = rsqrt(ss), c = softmax(z*alpha)
                # operates on batch slice [b0, b1)
                def softmax_update(it, b0, b1):
                    s = slice(b0 * NUM, b1 * NUM)
                    nw = (b1 - b0) * NUM
                    czs = cz[:, s]
                    if it == 0:
                        nc.vector.tensor_scalar_mul(czs, z_all[:, s], C0_BF)
                    else:
                        nc.vector.tensor_tensor(
                            czs, c_all[:, b0:b1, :].rearrange("p b i -> p (b i)"),
                            z_all[:, s], op=ALU.mult)
                    ps_ss = pss.tile([1, NBI], f32, tag="ss")
                    nc.tensor.matmul(ps_ss[:, 0:nw], onecol[:], czs,
                                     start=True, stop=True)
                    nc.vector.tensor_scalar_max(ssm[:, s], ps_ss[:, 0:nw], EPS)
                    nc.scalar.activation(ssm[:, s], ssm[:, s], AF.Sqrt)
                    nc.vector.reciprocal(alpha[:, s], ssm[:, s])
                    ps_al = pss.tile([128, NBI], f32, tag="al")
                    nc.tensor.matmul(ps_al[:, 0:nw], onerow[:], alpha[:, s],
                                     start=True, stop=True)
                    nc.vector.tensor_tensor(blog[:, s], z_all[:, s],
                                            ps_al[:, 0:nw], op=ALU.mult)
                    for b in range(b0, b1):
                        blv = blog[:, b * NUM:(b + 1) * NUM]
                        nc.vector.tensor_reduce(nmax[:, b:b + 1], blv,
                                                axis=mybir.AxisListType.X,
                                                op=ALU.max, negate=True)
                        nc.scalar.activation(eexp[:, b, :], blv, AF.Exp,
                                             bias=nmax[:, b:b + 1],
                                             accum_out=sume[:, b:b + 1])
                        nc.vector.reciprocal(rsum[:, b:b + 1], sume[:, b:b + 1])
                        nc.vector.tensor_scalar_mul(
                            c_all[:, b, :], eexp[:, b, :], rsum[:, b:b + 1])

                softmax_update(0, 0, B_LOC)

                # ---- it1: streams (PE) + z (V); creps upfront per batch (S) ----
                for b in range(B_LOC):
                    creps = []
                    for i in range(NUM):
                        crep = rt.tile([128, 128], bf16, tag="crep1", bufs=12,
                                       name=f"crep1_{b}_{i}")
                        nc.scalar.activation(crep[:], ones_bf[:], AF.Copy,
                                             scale=c_all[:, b, i:i + 1])
                        creps.append(crep)
                    for i in range(NUM):
                        pbc = psb.tile([128, JK], f32, tag="pbc")
                        o0 = i * JK
                        nc.tensor.matmul(pbc[:, 0:512], creps[i][:],
                                         u_hat[:, b, o0:o0 + 512],
                                         start=True, stop=True)
                        nc.tensor.matmul(pbc[:, 512:JK], creps[i][:],
                                         u_hat[:, b, o0 + 512:o0 + JK],
                                         start=True, stop=True)
                        un = b * NUM + i
                        nc.vector.scalar_tensor_tensor(
                            out=scr[:], in0=u_hat[:, b, o0:o0 + JK],
                            scalar=1.0, in1=pbc[:],
                            op0=ALU.mult, op1=ALU.mult,
                            accum_out=z_all[:, un:un + 1])

                # ---- it2 per batch: softmax(b) then final streams -> DMA out ----
                for b in range(B_LOC):
                    softmax_update(1, b, b + 1)
                    creps = []
                    for i in range(NUM):
                        crep = rt.tile([128, 128], bf16, tag="crep2", bufs=12,
                                       name=f"crep2_{b}_{i}")
                        nc.vector.tensor_scalar_mul(
                            crep[:], ones_bf[:], c_all[:, b, i:i + 1])
                        creps.append(crep)
                    for i in range(NUM):
                        pbc = psb.tile([128, JK], f32, tag="pbc")
                        o0 = i * JK
                        nc.tensor.matmul(pbc[:, 0:512], creps[i][:],
                                         u_hat[:, b, o0:o0 + 512],
                                         start=True, stop=True)
                        nc.tensor.matmul(pbc[:, 512:JK], creps[i][:],
                                         u_hat[:, b, o0 + 512:o0 + JK],
                                         start=True, stop=True)
                        ofin = rt.tile([1, JK], f32, tag="ofin", bufs=4)
                        nc.scalar.copy(ofin[:], pbc[0:1, :])
                        nc.sync.dma_start(out_d[b, i], ofin[:])
    nc.compile()
    return nc


_NC_CACHE = None


def _get_nc():
    global _NC_CACHE
    if _NC_CACHE is None:
        _NC_CACHE = build_nc()
    return _NC_CACHE


def _prep_inputs(u_vecs, W_conv, kernel):
    u_bf = np.ascontiguousarray(np.asarray(u_vecs, dtype=np.float32)).astype(BF)
    wc_bf = np.ascontiguousarray(np.asarray(W_conv, dtype=np.float32)).astype(BF)
    km_bf = np.ascontiguousarray(np.asarray(kernel, dtype=np.float32)).astype(BF)
    eye = np.eye(C, dtype=np.float32).astype(BF)
    return u_bf, wc_bf, km_bf, eye


def kernel(u_vecs, W_conv, kernel):
    u_bf, wc_bf, km_bf, eye = _prep_inputs(u_vecs, W_conv, kernel)
    nc = _get_nc()
    in_maps = [
        {"u": u_bf[ci * B_LOC:(ci + 1) * B_LOC], "wc": wc_bf, "km": km_bf, "eye": eye}
        for ci in range(N_CORES)
    ]
    res = run_bass_kernel_spmd(nc, in_maps, core_ids=list(range(N_CORES)))
    out = np.concatenate([r["out"] for r in res.results], axis=0)
    return out.reshape(B, NUM, D0, D1).astype(np.float32)
